# revision 11
# baseline (speedup 1.0000x reference)
"""Trainium2 Bass kernel for nn_Actor (topk_masking).

Pure data parallel across 8 NeuronCores: batch 16384 -> 2048 per core.
All weights replicated; BatchNorm folded into W1/W2 on the host.

Layout strategy: activations live feature-major ([feat partitions, batch free])
so every linear layer is weight-stationary matmul with the batch as the moving
operand (fp32, 512-wide). Head logits are PE-transposed to [batch, 64] tiles
where softmax/argmax/log-prob run on DVE/ACT in logit space (no divisions):
  select = argmax of filter-masked logits; logp = max - m - ln(sum(f*e^(l-m))).
The selected move row is fetched from DRAM with an indirect gather keyed by
on-chip argmax results. Weights ship as two host-packed blobs (one DMA each);
the two 512x512 target/promote trunk matrices stream per-use in row chunks.
"""

import numpy as np

B_FULL = 16384
N_CORES = 8
EPS = 1e-5


def _pack_chunks(W, K, M):
    """[K, M] -> [128, (K//128)*M] with chunk (k, m) at cols (k*(M//128)+m)*mc."""
    nk, nm = K // 128, max(M // 128, 1)
    mc = min(M, 128)
    return (W.reshape(nk, 128, nm, mc).transpose(1, 0, 2, 3)
            .reshape(128, nk * nm * mc).copy())


# (name, cols) in packing order; offsets computed identically at build time
PACK128 = [("w1", 2048), ("w2", 2048), ("ws1", 2048), ("ws2", 256),
           ("wt2", 256), ("wp2", 16), ("wp1st", 512),
           ("b1", 4), ("b2", 4), ("bs1", 4), ("bt1", 4), ("bp1", 4),
           ("ident", 128), ("iota_f", 64), ("wiota_f", 64), ("iota_p64", 1)]
PACK64 = [("wt1b", 512), ("bs2", 1), ("bt2", 1), ("bp2", 1)]


def _offsets(spec):
    out, c = {}, 0
    for name, w in spec:
        out[name] = c
        c += w
    return out, c


OFF128, C128 = _offsets(PACK128)
OFF64, C64 = _offsets(PACK64)


def _fold_bn(W, b, g, be, m, v):
    rs = (1.0 / np.sqrt(v + np.float32(EPS))).astype(np.float32)
    scale = (rs * g).astype(np.float32)
    return ((W * scale[None, :]).astype(np.float32),
            ((b - m) * scale + be).astype(np.float32))


def _prep_host(embeddings, teams, move_matrices, params):
    p = {k: np.asarray(v) for k, v in params.items()}
    W1, b1 = _fold_bn(np.asarray(p["W1"]), p["b1"], p["g1"], p["be1"], p["m1"], p["v1"])
    W2, b2 = _fold_bn(np.asarray(p["W2"]), p["b2"], p["g2"], p["be2"], p["m2"], p["v2"])

    parts128 = {
        "w1": _pack_chunks(W1, 512, 512),
        "w2": _pack_chunks(W2, 512, 512),
        "ws1": _pack_chunks(np.asarray(p["Ws1"]), 512, 512),
        "ws2": _pack_chunks(np.asarray(p["Ws2"]), 512, 64),
        "wt2": _pack_chunks(np.asarray(p["Wt2"]), 512, 64),
        "wp2": _pack_chunks(np.asarray(p["Wp2"]), 512, 4),
        "wp1st": np.concatenate([p["Wp1"][576:640], p["Wp1"][512:576]]).astype(np.float32),
        "b1": b1.reshape(4, 128).T,
        "b2": b2.reshape(4, 128).T,
        "bs1": np.asarray(p["bs1"]).reshape(4, 128).T,
        "bt1": np.asarray(p["bt1"]).reshape(4, 128).T,
        "bp1": np.asarray(p["bp1"]).reshape(4, 128).T,
        "ident": np.eye(128, dtype=np.float32),
        "iota_f": np.tile(np.arange(64, dtype=np.float32), (128, 1)),
        "wiota_f": np.tile(64.0 - np.arange(64, dtype=np.float32), (128, 1)),
        "iota_p64": (np.arange(128, dtype=np.float32) * 64.0).reshape(128, 1),
    }
    wpack128 = np.zeros((128, C128), np.float32)
    for name, w in PACK128:
        wpack128[:, OFF128[name]:OFF128[name] + w] = parts128[name]

    parts64 = {
        "wt1b": np.ascontiguousarray(p["Wt1"][512:576]),
        "bs2": np.asarray(p["bs2"]).reshape(64, 1),
        "bt2": np.asarray(p["bt2"]).reshape(64, 1),
        "bp2": np.pad(np.asarray(p["bp2"]).reshape(4, 1), ((0, 60), (0, 0))),
    }
    wpack64 = np.zeros((128, C64), np.float32)
    for name, w in PACK64:
        if name == "wt1b":
            wpack64[64:128, OFF64[name]:OFF64[name] + w] = parts64[name]
        else:
            wpack64[0:64, OFF64[name]:OFF64[name] + w] = parts64[name]

    wt1a = np.ascontiguousarray(p["Wt1"][:512], dtype=np.float32)
    wp1a = np.ascontiguousarray(p["Wp1"][:512], dtype=np.float32)

    x_cat = np.concatenate(
        [np.asarray(embeddings, np.float32), np.asarray(teams, np.float32)], axis=1)
    mm = np.asarray(move_matrices)
    assert mm.dtype == np.int32

    Bc = B_FULL // N_CORES
    in_maps = []
    for c in range(N_CORES):
        sl = slice(c * Bc, (c + 1) * Bc)
        in_maps.append({
            "xT": np.ascontiguousarray(x_cat[sl].T),
            "mmt": np.ascontiguousarray(mm[sl]).reshape(Bc * 64, 64),
            "wpack128": wpack128, "wpack64": wpack64,
            "wt1a": wt1a, "wp1a": wp1a,
        })
    return in_maps


# ---------------------------------------------------------------------------
# walrus wait-cap workaround (this walrus accepts 1 wait per instruction,
# 2 on EventSemaphore; Tile freely packs more)
# ---------------------------------------------------------------------------

def _make_fixed_tile_context():
    import concourse.mybir as mybir
    import concourse.tile as tile

    class FixedTileContext(tile.TileContext):
        def _drain_and_barrier(self, tick_clock, wait_clock):
            nc = self.nc
            probe = nc.sync.nop()
            wait_clock.add_sem_waits(
                probe.ins, tile.ScopedClock({None: tick_clock.global_clock}))
            si = probe.ins.sync_info
            if si is not None and len(si.on_wait) > 1:
                waits = list(si.on_wait)
                si.on_wait = [waits[0]]
                for w in waits[1:]:
                    extra = nc.sync.nop()
                    extra.ins.sync_info = mybir.SyncInfo(on_wait=[w], on_update=[])
            nc.sync.drain()
            nc.all_engine_barrier()
            assert self.sems is not None
            popped = nc._tile_sem_poison_stack.pop()
            assert popped is self._sem_poison
            nc.clear_and_free_semaphores(list(self.sems.allocated().values()))
            nc.all_engine_barrier()

    return FixedTileContext


def _legalize_waits(nc):
    import concourse.mybir as mybir

    counter = [0]
    for f in nc.m.functions:
        blocks = f.blocks
        for bi in range(len(blocks)):
            bb = blocks[bi]
            new_insts = []
            changed = False
            for inst in bb.instructions:
                si = inst.sync_info
                cap = 2 if isinstance(inst, mybir.InstEventSemaphore) else 1
                if si is not None and len(si.on_wait) > cap:
                    waits = list(si.on_wait)
                    for w in waits[: len(waits) - cap]:
                        nop = mybir.InstNoOp(
                            name=f"I-wsplit-{counter[0]}", ins=[], outs=[])
                        counter[0] += 1
                        nop.engine = inst.engine
                        nop.sync_info = mybir.SyncInfo(on_wait=[w], on_update=[])
                        new_insts.append(nop)
                    si.on_wait = waits[len(waits) - cap:]
                    changed = True
                new_insts.append(inst)
            if changed:
                nb = mybir.BasicBlock(name=bb.name, instructions=new_insts)
                nb.IsExit = bb.IsExit
                nb.IsLoopEntry = bb.IsLoopEntry
                nb.IsPredicated = bb.IsPredicated
                blocks[bi] = nb


# ---------------------------------------------------------------------------
# device program
# ---------------------------------------------------------------------------

def build_program(Bc=B_FULL // N_CORES, BT=512, legalize=True):
    import contextlib

    import concourse.bass as bass
    import concourse.mybir as mybir
    from concourse.bass import IndirectOffsetOnAxis

    f32 = mybir.dt.float32
    i32 = mybir.dt.int32
    AF = mybir.ActivationFunctionType
    ALU = mybir.AluOpType
    AX = mybir.AxisListType

    NBT = Bc // BT
    NT = Bc // 128
    TPB = BT // 128

    FixedTileContext = _make_fixed_tile_context()
    nc = bass.Bass("TRN2", target_bir_lowering=False, debug=False,
                   enable_asserts=True, num_devices=N_CORES)

    def din(name, shape, dt=f32):
        return nc.dram_tensor(name, shape, dt, kind="ExternalInput").ap()

    xT = din("xT", [512, Bc])
    mmt = din("mmt", [Bc * 64, 64], i32)
    dwp128 = din("wpack128", [128, C128])
    dwp64 = din("wpack64", [128, C64])
    dwt1a = din("wt1a", [512, 512])
    dwp1a = din("wp1a", [512, 512])

    sel_out = nc.dram_tensor("sel_out", [Bc, 1], i32, kind="ExternalOutput").ap()
    tgt_out = nc.dram_tensor("tgt_out", [Bc, 1], i32, kind="ExternalOutput").ap()
    pro_out = nc.dram_tensor("pro_out", [Bc, 1], i32, kind="ExternalOutput").ap()
    logp_out = nc.dram_tensor("logp_out", [Bc, 1], f32, kind="ExternalOutput").ap()

    with FixedTileContext(nc) as tc:
        ctx = contextlib.ExitStack()
        with ctx:
            wpool = ctx.enter_context(tc.tile_pool(name="wpool", bufs=1))
            actp = ctx.enter_context(tc.tile_pool(name="actp", bufs=8))
            projp = ctx.enter_context(tc.tile_pool(name="projp", bufs=1))
            movep = ctx.enter_context(tc.tile_pool(name="movep", bufs=2))
            filtp = ctx.enter_context(tc.tile_pool(name="filtp", bufs=16))
            slbp = ctx.enter_context(tc.tile_pool(name="slbp", bufs=2))
            ohp = ctx.enter_context(tc.tile_pool(name="ohp", bufs=1))
            rowp = ctx.enter_context(tc.tile_pool(name="rowp", bufs=1))
            ephp = ctx.enter_context(tc.tile_pool(name="ephp", bufs=2))
            stgp = ctx.enter_context(tc.tile_pool(name="stgp", bufs=1))
            wstr = ctx.enter_context(tc.tile_pool(name="wstr", bufs=4))
            pmm = ctx.enter_context(tc.tile_pool(name="pmm", bufs=4, space="PSUM"))
            phead = ctx.enter_context(tc.tile_pool(name="phead", bufs=2,
                                                   space="PSUM"))
            ptr = ctx.enter_context(tc.tile_pool(name="ptr", bufs=2, space="PSUM"))

            # ---- packed weights / constants: two DMAs ----
            wp128 = wpool.tile([128, C128], f32, name="wp128")
            nc.sync.dma_start(out=wp128[:], in_=dwp128[:])
            wp64 = wpool.tile([128, C64], f32, name="wp64")
            nc.sync.dma_start(out=wp64[:], in_=dwp64[:])

            def w128(name, k, m, nm=4, mc=128):
                off = OFF128[name] + (k * nm + m) * mc
                return wp128[:, off:off + mc]

            def bias128(name, m):
                off = OFF128[name] + m
                return wp128[:, off:off + 1]

            def w64(name, m, mc=128):
                off = OFF64[name] + m * mc
                return wp64[64:128, off:off + mc]

            tid = wp128[:, OFF128["ident"]:OFF128["ident"] + 128]
            tiota = wp128[:, OFF128["iota_f"]:OFF128["iota_f"] + 64]
            twiota = wp128[:, OFF128["wiota_f"]:OFF128["wiota_f"] + 64]
            tiop64 = wp128[:, OFF128["iota_p64"]:OFF128["iota_p64"] + 1]

            tnegbig = wpool.tile([128, 64], f32, name="tnegbig")
            nc.vector.memset(tnegbig[:], -1e30)
            tc64 = wpool.tile([128, 1], f32, name="tc64")
            nc.vector.memset(tc64[:], 64.0)

            # ---- x input ----
            xc = []
            for k in range(4):
                t = actp.tile([128, Bc], f32, name=f"x{k}", tag="act")
                nc.sync.dma_start(out=t[:], in_=xT[k*128:(k+1)*128, :])
                xc.append(t)

            # ---- dense layer over resident packed weights ----
            def dense(in_fn, wname, nk, bname, out_fn, relu, mp=128, nm=4):
                for b in range(NBT):
                    cols = slice(b * BT, (b + 1) * BT)
                    for m in range(nm):
                        ps = pmm.tile([mp, BT], f32, name="ps_mm", tag="pmm",
                                      space="PSUM")
                        for k in range(nk):
                            nc.tensor.matmul(ps[:], w128(wname, k, m, nm=nm, mc=mp),
                                             in_fn(k, b, cols),
                                             start=(k == 0), stop=(k == nk - 1))
                        nc.scalar.activation(out_fn(m, b, cols), ps[:],
                                             AF.Relu if relu else AF.Identity,
                                             bias=bias128(bname, m), scale=1.0)

            def mk_acts(name):
                return [actp.tile([128, Bc], f32, name=f"{name}{m}", tag="act")
                        for m in range(4)]

            hc = mk_acts("h")
            dense(lambda k, b, c: xc[k][:, c], "w1", 4, "b1",
                  lambda m, b, c: hc[m][:, c], True)

            # ---- move-matrix reduction stream (emitted after L1 so its DMAs
            # don't delay trunk startup; it has lots of slack) ----
            filt = []
            for t in range(NT):
                src = mmt.rearrange("(t p s) j -> t p (s j)", t=NT, p=128)
                ft = filtp.tile([128, 64], f32, name=f"ft{t}", tag="ft")
                for h in range(2):
                    mv = movep.tile([128, 2048], i32, name=f"mv{t}_{h}", tag="mv")
                    nc.sync.dma_start(out=mv[:], in_=src[t, :, h*2048:(h+1)*2048])
                    s_h = ephp.tile([128, 32], i32, name=f"s_{h}", tag=f"s_{h}")
                    with nc.allow_low_precision(reason="int32 sums <= 128"):
                        nc.vector.reduce_sum(
                            s_h[:], mv[:].rearrange("p (s j) -> p s j", j=64),
                            axis=AX.X)
                    nc.vector.tensor_scalar(out=ft[:, h*32:(h+1)*32], in0=s_h[:],
                                            scalar1=0, scalar2=None, op0=ALU.is_gt)
                filt.append(ft)

            projc = [projp.tile([128, Bc], f32, name=f"proj{m}") for m in range(4)]
            dense(lambda k, b, c: hc[k][:, c], "w2", 4, "b2",
                  lambda m, b, c: projc[m][:, c], True)

            s1c = mk_acts("s1")
            dense(lambda k, b, c: projc[k][:, c], "ws1", 4, "bs1",
                  lambda m, b, c: s1c[m][:, c], True)

            # ---- streamed dense for T1/P1 (weights from DRAM in row chunks) ----
            def dense_streamed(dram_w, extra, bname, out_fn):
                """psum[m] accumulates 4 streamed k-rows of dram_w, then the
                extra (w_fn, in_fn, kp) chunk, then ACT-drains with relu."""
                w_fn, in_ap_fn = extra
                for b in range(NBT):
                    cols = slice(b * BT, (b + 1) * BT)
                    pss = [pmm.tile([128, BT], f32, name="ps_s", tag="pmm",
                                    space="PSUM") for _ in range(4)]
                    for k in range(4):
                        wrow = wstr.tile([128, 512], f32, name="wrow", tag="wrow")
                        nc.sync.dma_start(out=wrow[:],
                                          in_=dram_w[k*128:(k+1)*128, :])
                        for m in range(4):
                            nc.tensor.matmul(pss[m][:], wrow[:, m*128:(m+1)*128],
                                             projc[k][:, cols],
                                             start=(k == 0), stop=False)
                    for m in range(4):
                        nc.tensor.matmul(pss[m][:], w_fn(m), in_ap_fn(b, cols),
                                         start=False, stop=True)
                    for m in range(4):
                        nc.scalar.activation(out_fn(m, b, cols), pss[m][:], AF.Relu,
                                             bias=bias128(bname, m), scale=1.0)

            # ---- head helpers ----
            def transpose_to(in_ap, kdim, width, pslice):
                tp = ptr.tile([128, 128], f32, name="tp", tag="tp", space="PSUM")
                out_ap = tp[pslice, 0:width]
                nc.tensor.matmul(out_ap, in_ap, tid[0:kdim, 0:kdim],
                                 is_transpose=True, start=True, stop=True)
                return out_ap

            def argmax_free(fl, width, name):
                fm = ephp.tile([128, 1], f32, name=f"fm_{name}", tag=f"fm_{name}")
                nc.vector.tensor_reduce(fm[:], fl[:], axis=AX.X, op=ALU.max)
                eq = ephp.tile([128, width], f32, name=f"eq_{name}",
                               tag=f"eq_{name}")
                nc.vector.tensor_scalar(out=eq[:], in0=fl[:], scalar1=fm[:],
                                        scalar2=None, op0=ALU.is_equal)
                sc = ephp.tile([128, width], f32, name=f"sc_{name}",
                               tag=f"sc_{name}")
                nc.vector.tensor_tensor(out=sc[:], in0=eq[:],
                                        in1=twiota[:, 0:width], op=ALU.mult)
                ms = ephp.tile([128, 1], f32, name=f"ms_{name}", tag=f"ms_{name}")
                nc.vector.tensor_reduce(ms[:], sc[:], axis=AX.X, op=ALU.max)
                idx = ephp.tile([128, 1], f32, name=f"idx_{name}",
                                tag=f"idx_{name}")
                nc.vector.scalar_tensor_tensor(out=idx[:], in0=ms[:], scalar=-1.0,
                                               in1=tc64[:], op0=ALU.mult,
                                               op1=ALU.add)
                return idx, fm

            def logits_bt(chunk_ap, t, width, name):
                col = (t % TPB) * 128
                src = transpose_to(chunk_ap[:, col:col+128], width, width,
                                   slice(0, 128))
                lb = ephp.tile([128, width], f32, name=f"lb_{name}",
                               tag=f"lb_{name}")
                nc.vector.tensor_copy(lb[:], src)
                return lb

            def softmax_logp(lb, weight_ap, width, name):
                negm = ephp.tile([128, 1], f32, name=f"nm_{name}", tag=f"nm_{name}")
                nc.vector.tensor_reduce(negm[:], lb[:], axis=AX.X, op=ALU.max,
                                        negate=True)
                ex = ephp.tile([128, width], f32, name=f"ex_{name}",
                               tag=f"ex_{name}")
                nc.scalar.activation(ex[:], lb[:], AF.Exp, bias=negm[:], scale=1.0)
                scr = ephp.tile([128, width], f32, name=f"scr_{name}",
                                tag=f"scr_{name}")
                nc.vector.tensor_tensor(out=scr[:], in0=ex[:], in1=weight_ap,
                                        op=ALU.mult)
                sv = ephp.tile([128, 1], f32, name=f"sv_{name}", tag=f"sv_{name}")
                nc.vector.reduce_sum(sv[:], scr[:], axis=AX.X)
                lns = ephp.tile([128, 1], f32, name=f"ls_{name}", tag=f"ls_{name}")
                nc.scalar.activation(lns[:], sv[:], AF.Ln)
                return negm, lns

            # persistent head state
            ohstT = [ohp.tile([128, BT], f32, name=f"ohstT{b}")
                     for b in range(NBT)]
            rowf = [rowp.tile([128, 64], f32, name=f"rowf{t}") for t in range(NT)]
            st_sel = stgp.tile([128, NT], f32, name="st_sel")
            st_tgt = stgp.tile([128, NT], f32, name="st_tgt")
            st_pro = stgp.tile([128, NT], f32, name="st_pro")
            st_logp = stgp.tile([128, NT], f32, name="st_logp")

            # ---- S2 + select heads, interleaved per moving tile ----
            slc = [slbp.tile([64, BT], f32, name=f"sl{b}", tag="slb")
                   for b in range(NBT)]
            for b in range(NBT):
                cols = slice(b * BT, (b + 1) * BT)
                ps = phead.tile([64, BT], f32, name="ps_h", tag="phead",
                                space="PSUM")
                for k in range(4):
                    nc.tensor.matmul(ps[:], w128("ws2", k, 0, nm=1, mc=64),
                                     s1c[k][:, cols], start=(k == 0), stop=(k == 3))
                nc.scalar.activation(slc[b][:], ps[:], AF.Identity,
                                     bias=wp64[0:64, OFF64["bs2"]:OFF64["bs2"]+1],
                                     scale=1.0)
                for t in range(b * TPB, (b + 1) * TPB):
                    col = (t % TPB) * 128
                    slb = logits_bt(slc[b], t, 64, "s")
                    fl = ephp.tile([128, 64], f32, name="fl_s", tag="fl_s")
                    nc.vector.tensor_copy(fl[:], tnegbig[:])
                    nc.vector.copy_predicated(fl[:], filt[t][:].bitcast(i32),
                                              slb[:])
                    negm, lns = softmax_logp(slb, filt[t][:], 64, "s")
                    idx, fm = argmax_free(fl, 64, "s")
                    nc.vector.tensor_copy(st_sel[:, t:t+1], idx[:])
                    a0 = ephp.tile([128, 1], f32, name="a0_s", tag="a0_s")
                    nc.vector.tensor_tensor(out=a0[:], in0=fm[:], in1=negm[:],
                                            op=ALU.add)
                    nc.vector.tensor_tensor(out=st_logp[:, t:t+1], in0=a0[:],
                                            in1=lns[:], op=ALU.subtract)
                    ohs = ephp.tile([128, 64], f32, name="ohs", tag="ohs")
                    nc.vector.tensor_scalar(out=ohs[:], in0=tiota[:],
                                            scalar1=idx[:], scalar2=None,
                                            op0=ALU.is_equal)
                    src = transpose_to(ohs[:], 128, 128, slice(0, 64))
                    ohtmp = ephp.tile([64, 128], f32, name="ohtmp", tag="ohtmp")
                    nc.vector.tensor_copy(ohtmp[:], src)
                    nc.sync.dma_start(out=ohstT[b][64:128, col:col+128],
                                      in_=ohtmp[:])
                    gi = ephp.tile([128, 1], f32, name="gi", tag="gi")
                    nc.vector.scalar_tensor_tensor(out=gi[:], in0=tiop64[:],
                                                   scalar=float(t * 8192),
                                                   in1=idx[:], op0=ALU.add,
                                                   op1=ALU.add)
                    gii = ephp.tile([128, 1], i32, name="gii", tag="gii")
                    nc.vector.tensor_copy(gii[:], gi[:])
                    rowi = ephp.tile([128, 64], i32, name="rowi", tag="rowi")
                    nc.gpsimd.indirect_dma_start(
                        out=rowi[:], out_offset=None, in_=mmt[:],
                        in_offset=IndirectOffsetOnAxis(ap=gii[:, :1], axis=0))
                    nc.vector.tensor_copy(rowf[t][:], rowi[:])

            # ---- T1 + T2 + target heads ----
            tlc = [slbp.tile([64, BT], f32, name=f"tl{b}", tag="tlb")
                   for b in range(NBT)]
            t1c = mk_acts("t1")
            dense_streamed(dwt1a,
                           (lambda m: w64("wt1b", m),
                            lambda b, c: ohstT[b][64:128, :]),
                           "bt1", lambda m, b, c: t1c[m][:, c])
            for b in range(NBT):
                cols = slice(b * BT, (b + 1) * BT)
                ps = phead.tile([64, BT], f32, name="ps_h", tag="phead",
                                space="PSUM")
                for k in range(4):
                    nc.tensor.matmul(ps[:], w128("wt2", k, 0, nm=1, mc=64),
                                     t1c[k][:, cols], start=(k == 0), stop=(k == 3))
                nc.scalar.activation(tlc[b][:], ps[:], AF.Identity,
                                     bias=wp64[0:64, OFF64["bt2"]:OFF64["bt2"]+1],
                                     scale=1.0)
                for t in range(b * TPB, (b + 1) * TPB):
                    col = (t % TPB) * 128
                    tlb = logits_bt(tlc[b], t, 64, "t")
                    rmask = ephp.tile([128, 64], i32, name="rmask", tag="rmask")
                    nc.vector.tensor_scalar(out=rmask[:], in0=rowf[t][:],
                                            scalar1=0.0, scalar2=None,
                                            op0=ALU.is_gt)
                    rm = ephp.tile([128, 64], f32, name="rm", tag="rm")
                    nc.vector.tensor_scalar_max(rm[:], rowf[t][:], 0.5)
                    lr = ephp.tile([128, 64], f32, name="lr", tag="lr")
                    nc.scalar.activation(lr[:], rm[:], AF.Ln)
                    fl0 = ephp.tile([128, 64], f32, name="fl0", tag="fl0")
                    nc.vector.tensor_tensor(out=fl0[:], in0=tlb[:], in1=lr[:],
                                            op=ALU.add)
                    fl = ephp.tile([128, 64], f32, name="fl_t", tag="fl_t")
                    nc.vector.tensor_copy(fl[:], tnegbig[:])
                    nc.vector.copy_predicated(fl[:], rmask[:], fl0[:])
                    negm, lns = softmax_logp(tlb, rowf[t][:], 64, "t")
                    idx, fm = argmax_free(fl, 64, "t")
                    nc.vector.tensor_copy(st_tgt[:, t:t+1], idx[:])
                    a0 = ephp.tile([128, 1], f32, name="a0_t", tag="a0_t")
                    nc.vector.tensor_tensor(out=a0[:], in0=fm[:], in1=negm[:],
                                            op=ALU.add)
                    a1 = ephp.tile([128, 1], f32, name="a1_t", tag="a1_t")
                    nc.vector.tensor_tensor(out=a1[:], in0=a0[:], in1=lns[:],
                                            op=ALU.subtract)
                    nc.vector.tensor_tensor(out=st_logp[:, t:t+1],
                                            in0=st_logp[:, t:t+1], in1=a1[:],
                                            op=ALU.add)
                    oht = ephp.tile([128, 64], f32, name="oht", tag="oht")
                    nc.vector.tensor_scalar(out=oht[:], in0=tiota[:],
                                            scalar1=idx[:], scalar2=None,
                                            op0=ALU.is_equal)
                    src = transpose_to(oht[:], 128, 128, slice(0, 64))
                    nc.vector.tensor_copy(ohstT[b][0:64, col:col+128], src)
                    pv = ephp.tile([128, 1], f32, name="pv", tag="pv")
                    scr2 = ephp.tile([128, 64], f32, name="scr2", tag="scr2")
                    nc.vector.tensor_tensor(out=scr2[:], in0=rowf[t][:],
                                            in1=oht[:], op=ALU.mult)
                    nc.vector.reduce_sum(pv[:], scr2[:], axis=AX.X)
                    pf = filtp.tile([128, 1], f32, name=f"pf{t}", tag="pf")
                    nc.vector.tensor_scalar(out=pf[:], in0=pv[:], scalar1=2.0,
                                            scalar2=None, op0=ALU.is_equal)
                    filt.append(pf)

            # ---- P1 + P2 + promote heads ----
            plc = [slbp.tile([4, BT], f32, name=f"pl{b}", tag="plb")
                   for b in range(NBT)]
            p1c = mk_acts("p1")
            dense_streamed(dwp1a,
                           (lambda m: w128("wp1st", 0, m, nm=4),
                            lambda b, c: ohstT[b][:]),
                           "bp1", lambda m, b, c: p1c[m][:, c])
            for b in range(NBT):
                cols = slice(b * BT, (b + 1) * BT)
                ps = phead.tile([4, BT], f32, name="ps_h", tag="phead",
                                space="PSUM")
                for k in range(4):
                    nc.tensor.matmul(ps[:], w128("wp2", k, 0, nm=1, mc=4),
                                     p1c[k][:, cols], start=(k == 0), stop=(k == 3))
                nc.scalar.activation(plc[b][:], ps[:], AF.Identity,
                                     bias=wp64[0:4, OFF64["bp2"]:OFF64["bp2"]+1],
                                     scale=1.0)
                for t in range(b * TPB, (b + 1) * TPB):
                    plb = logits_bt(plc[b], t, 4, "p")
                    pf = filt[NT + t]
                    negm3 = ephp.tile([128, 1], f32, name="nm_p", tag="nm_p")
                    nc.vector.tensor_reduce(negm3[:], plb[:], axis=AX.X,
                                            op=ALU.max, negate=True)
                    ex3 = ephp.tile([128, 4], f32, name="ex_p", tag="ex_p")
                    s3 = ephp.tile([128, 1], f32, name="s3", tag="s3")
                    nc.scalar.activation(ex3[:], plb[:], AF.Exp, bias=negm3[:],
                                         scale=1.0, accum_out=s3[:])
                    ls3 = ephp.tile([128, 1], f32, name="ls3", tag="ls3")
                    nc.scalar.activation(ls3[:], s3[:], AF.Ln)
                    plp = ephp.tile([128, 1], f32, name="plp", tag="plp")
                    nc.vector.scalar_tensor_tensor(out=plp[:], in0=ls3[:],
                                                   scalar=-1.0, in1=pf[:],
                                                   op0=ALU.mult, op1=ALU.mult)
                    nc.vector.tensor_tensor(out=st_logp[:, t:t+1],
                                            in0=st_logp[:, t:t+1], in1=plp[:],
                                            op=ALU.add)
                    idx, _fm = argmax_free(plb, 4, "p")
                    pr = ephp.tile([128, 1], f32, name="pr", tag="pr")
                    nc.vector.scalar_tensor_tensor(out=pr[:], in0=idx[:],
                                                   scalar=1.0, in1=pf[:],
                                                   op0=ALU.add, op1=ALU.mult)
                    nc.vector.tensor_scalar(out=st_pro[:, t:t+1], in0=pr[:],
                                            scalar1=1.0, scalar2=None,
                                            op0=ALU.subtract)

            # ---- cast + DMA outputs ----
            def emit_out(stage, dram, dt):
                cast = stgp.tile([128, NT], dt, name=f"cast_{dram.tensor.name}")
                nc.vector.tensor_copy(cast[:], stage[:])
                dst = dram.rearrange("(t p) one -> p t one", p=128)
                nc.sync.dma_start(out=dst[:, :, 0], in_=cast[:])

            emit_out(st_sel, sel_out, i32)
            emit_out(st_tgt, tgt_out, i32)
            emit_out(st_pro, pro_out, i32)
            emit_out(st_logp, logp_out, f32)

    if legalize:
        _legalize_waits(nc)
    return nc


_CACHED = {}


def kernel(embeddings, teams, move_matrices, params):
    from concourse.bass_utils import run_bass_kernel_spmd

    in_maps = _prep_host(embeddings, teams, move_matrices, params)
    if "nc" not in _CACHED:
        _CACHED["nc"] = build_program()
    nc = _CACHED["nc"]
    res = run_bass_kernel_spmd(nc, in_maps, core_ids=list(range(N_CORES)))
    sel = np.concatenate([r["sel_out"] for r in res.results]).astype(np.int32)
    tgt = np.concatenate([r["tgt_out"] for r in res.results]).astype(np.int32)
    pro = np.concatenate([r["pro_out"] for r in res.results]).astype(np.int32)
    logp = np.concatenate([r["logp_out"] for r in res.results]).astype(np.float32)
    return sel, tgt, pro, logp
